# revision 1
# baseline (speedup 1.0000x reference)
"""Trainium2 Bass kernel for BackendQueryPooling.

Math simplifications used (all exact in exact arithmetic):
  - The k-projection folds into the shared query:
        scores[b,l,h] = x[b,l] . qw[h],  qw[h] = (q_h @ wk_head_h) / sqrt(hd)
    (the bk contribution is constant per h and cancels in softmax).
  - v is never materialized:
        ctx[n,h,:] = (sum_l w[n,h,l] x[b,l]) @ wvT_head + bv_head
    since sum_l w = 1 after normalization.
  - Per-(n,h) softmax uses unnormalized exp (scores are O(0.2), no overflow)
    with a multiplicative 0/1 mask; normalization by the sum computed via an
    appended ones-column in the same matmul.
  - Backends with no tokens produce 0 exactly as the reference does (the
    reference falls back to the full mask but then zeroes by has_tokens).

Sharding: data-parallel over batch. 16 batches / 8 cores = 2 batches per core.
No collectives; outputs are disjoint.
"""

import os
import sys

sys.path.insert(0, "/opt/trn_rl_repo")

import numpy as np
import ml_dtypes

import concourse.bass as bass
import concourse.bacc as bacc
import concourse.tile as tile
from concourse import mybir
from concourse.bass_utils import run_bass_kernel_spmd

BF16 = ml_dtypes.bfloat16
FP8 = ml_dtypes.float8_e4m3
QW_SHIFT = 13  # qw values ~7e-4 underflow fp8; pre-scale by 2**13, undo in exp scale
F32 = np.float32

B, L, D = 16, 8192, 256
H, HD, NB = 8, 32, 8
NCORES = 8
BPC = B // NCORES          # batches per core
NCH = L // 128             # 64 l-chunks of 128
SCALE = 1.0 / np.sqrt(HD)
NH = NB * H                # 64 (n, h) pairs
JW = D + 1                 # 257: x chunk width incl. ones column

_CACHE = {}
LAST_RESULT = None


def _patched_act_tables():
    """Make the act-table chooser land Exp AND Ln in one set
    (natural_log_exp_and_others) instead of thrashing between
    exp_and_others and natural_log (1.28us reload per switch).
    Order/size of the dict is preserved so act_func_set_ids stay valid."""
    from concourse.hw_specs import get_activation_tables

    AF = mybir.ActivationFunctionType

    def patched(arch):
        t = {k: set(v) for k, v in get_activation_tables(arch).items()}
        for name in t:
            if name != "natural_log_exp_and_others":
                t[name].discard(AF.Exp)
                t[name].discard(AF.Ln)
        return t

    return patched


def _build_nc(zero_bv=True, zero_br=True):
    nc = bacc.Bacc("TRN2", target_bir_lowering=False)
    dt = mybir.dt

    PIECES = [20, 20, 16, 8]   # l-chunks per pipeline piece (small last)
    NP = len(PIECES)
    OFFS = [sum(PIECES[:k]) for k in range(NP + 1)]

    xT_d = nc.dram_tensor("xT", [BPC, 2, 128, L], dt.float8e4, kind="ExternalInput")
    xe_d = nc.dram_tensor("xext", [BPC, 128, NCH * JW], dt.bfloat16, kind="ExternalInput")
    m_d = nc.dram_tensor("m", [BPC, 128, NCH * NB], dt.bfloat16, kind="ExternalInput")
    qwT_d = nc.dram_tensor("qwT", [2, 128, H], dt.float8e4, kind="ExternalInput")
    wvT_d = nc.dram_tensor("wvT", [2, 128, D], dt.float32, kind="ExternalInput")
    woT_d = nc.dram_tensor("woT", [2, 128, D], dt.float32, kind="ExternalInput")
    gf_d = nc.dram_tensor("gf", [BPC, NB, D], dt.float32, kind="ExternalInput")
    bf_d = nc.dram_tensor("bf", [BPC, NB, D], dt.float32, kind="ExternalInput")
    br_d = nc.dram_tensor("br", [NB, D], dt.float32, kind="ExternalInput")
    bv_d = nc.dram_tensor("bv", [128, 2], dt.float32, kind="ExternalInput")
    id_d = nc.dram_tensor("ident", [128, 128], dt.float32, kind="ExternalInput")
    out_d = nc.dram_tensor("out", [BPC, NB, D], dt.float32, kind="ExternalOutput")

    with tile.TileContext(nc) as tc:
        with (
            tc.tile_pool(name="consts", bufs=1) as consts,
            tc.tile_pool(name="big", bufs=2) as big,
            tc.tile_pool(name="work", bufs=2) as work,
            tc.tile_pool(name="psc", bufs=3, space="PSUM") as psc,
            tc.tile_pool(name="psy", bufs=1, space="PSUM") as psy,
            tc.tile_pool(name="pst", bufs=2, space="PSUM") as pst,
        ):
            # qwT is on the scores critical path: load first
            qwT_sb = consts.tile([128, 2, H], dt.float8e4)
            nc.sync.dma_start(out=qwT_sb, in_=qwT_d[:].rearrange("c p h -> p c h"))

            def load_tail_consts():
                # needed only from the first batch's tail onward
                wvT_sb = consts.tile([128, 2, D], dt.float32)
                nc.sync.dma_start(out=wvT_sb, in_=wvT_d[:].rearrange("c p e -> p c e"))
                woT_sb = consts.tile([128, 2, D], dt.float32)
                nc.sync.dma_start(out=woT_sb, in_=woT_d[:].rearrange("c p e -> p c e"))
                br_sb = consts.tile([NB, D], dt.float32)
                nc.sync.dma_start(out=br_sb, in_=br_d[:])
                bv_sb = consts.tile([128, 2], dt.float32)
                nc.sync.dma_start(out=bv_sb, in_=bv_d[:])
                id_sb = consts.tile([128, 128], dt.float32)
                nc.sync.dma_start(out=id_sb, in_=id_d[:])
                eps_sb = consts.tile([128, 1], dt.float32)
                nc.vector.memset(eps_sb, 1e-5)
                return wvT_sb, woT_sb, br_sb, bv_sb, id_sb, eps_sb

            tail_consts = None
            for i in range(BPC):
                # ---- loads, in consumption order ----
                m_sb = work.tile([128, NCH, NB], dt.bfloat16, tag="m")
                nc.sync.dma_start(
                    out=m_sb, in_=m_d[i].rearrange("p (c n) -> p c n", n=NB)
                )
                xT_sb, xe_sb = {}, {}
                # xT is small in fp8: load each d-chunk as one 1 MiB DMA
                xT_full = {}
                for dc in range(2):
                    t = big.tile([128, L], dt.float8e4, tag=f"xTf{dc}")
                    nc.sync.dma_start(out=t, in_=xT_d[i, dc])
                    xT_full[dc] = t
                for q in range(NP):
                    pc = PIECES[q]
                    for dc in range(2):
                        xT_sb[(dc, q)] = xT_full[dc][:, OFFS[q] * 128:OFFS[q + 1] * 128]
                    xe = big.tile([128, pc, JW], dt.bfloat16, tag=f"xe{q}")
                    nc.sync.dma_start(
                        out=xe,
                        in_=xe_d[i, :, OFFS[q] * JW:OFFS[q + 1] * JW].rearrange(
                            "p (c j) -> p c j", j=JW
                        ),
                    )
                    xe_sb[q] = xe
                    if i == 0 and q == 1:
                        tail_consts = load_tail_consts()
                g_sb = work.tile([NB, D], dt.float32, tag="g")
                nc.sync.dma_start(out=g_sb, in_=gf_d[i])
                b_sb = work.tile([NB, D], dt.float32, tag="b")
                nc.sync.dma_start(out=b_sb, in_=bf_d[i])
                wvT_sb, woT_sb, br_sb, bv_sb, id_sb, eps_sb = tail_consts

                # ---- pipelined pieces: scores -> exp -> w -> y ----
                y_ps = psy.tile([NH, JW], dt.float32, tag="y")

                def scores_piece(q):
                    pc = PIECES[q]
                    sc_ps = psc.tile([128, pc * H], dt.float32, tag="sc")
                    for cl in range(pc):
                        for dc in range(2):
                            nc.tensor.matmul(
                                sc_ps[:, cl * 8:(cl + 1) * 8],
                                lhsT=xT_sb[(dc, q)][:, cl * 128:(cl + 1) * 128],
                                rhs=qwT_sb[:, dc, :],
                                start=(dc == 0),
                                stop=(dc == 1),
                            )
                    E_sb = work.tile([128, pc, H], dt.bfloat16, tag=f"E{q}")
                    nc.scalar.activation(
                        out=E_sb.rearrange("p c h -> p (c h)"),
                        in_=sc_ps,
                        func=mybir.ActivationFunctionType.Exp,
                        scale=float(2.0 ** -QW_SHIFT),
                    )
                    w = big.tile([128, pc, NB, H], dt.bfloat16, tag=f"w{q}")
                    nc.vector.tensor_tensor(
                        out=w,
                        in0=E_sb.unsqueeze(2).broadcast_to([128, pc, NB, H]),
                        in1=m_sb[:, OFFS[q]:OFFS[q + 1]].unsqueeze(3).broadcast_to(
                            [128, pc, NB, H]
                        ),
                        op=mybir.AluOpType.mult,
                    )
                    return w

                def y_piece(q, w):
                    for cl in range(PIECES[q]):
                        nc.tensor.matmul(
                            y_ps,
                            lhsT=w[:, cl],
                            rhs=xe_sb[q][:, cl],
                            start=(q == 0 and cl == 0),
                            stop=(q == NP - 1 and cl == PIECES[q] - 1),
                        )

                w_prev = scores_piece(0)
                for q in range(1, NP):
                    w_cur = scores_piece(q)
                    y_piece(q - 1, w_prev)
                    w_prev = w_cur
                y_piece(NP - 1, w_prev)

                # ---- normalize ----
                s_sb = work.tile([NH, 1], dt.float32, tag="s")
                nc.vector.tensor_scalar_add(s_sb, y_ps[:, D:JW], 1e-30)
                r_sb = work.tile([NH, 1], dt.float32, tag="r")
                nc.vector.reciprocal(r_sb, s_sb)
                yn_sb = work.tile([NH, D], dt.float32, tag="yn")
                nc.vector.tensor_scalar_mul(yn_sb, y_ps[:, 0:D], r_sb)

                # ---- transpose y_norm -> yT [d, nh] (2 chunks) ----
                yT_sb = work.tile([128, 2, NH], dt.float32, tag="yT")
                for ec in range(2):
                    tr_ps = pst.tile([128, NH], dt.float32, tag="tail")
                    nc.tensor.transpose(
                        tr_ps, yn_sb[:, ec * 128:(ec + 1) * 128], id_sb[0:NH, 0:NH]
                    )
                    nc.vector.tensor_copy(yT_sb[:, ec, :], tr_ps)

                # ---- ctxT[e', (t, n)] via per-head matmuls ----
                cx_ps = pst.tile([128, 2 * NB], dt.float32, tag="tail")
                for h in range(H):
                    t, r4 = divmod(h, 4)
                    r0 = r4 * 32
                    for dc in range(2):
                        nc.tensor.matmul(
                            cx_ps[r0:r0 + 32, t * NB:(t + 1) * NB],
                            lhsT=wvT_sb[:, dc, 32 * h:32 * h + 32],
                            rhs=yT_sb[:, dc, :].rearrange("p (n h2) -> p n h2", h2=H)[:, :, h],
                            start=(dc == 0),
                            stop=(dc == 1),
                            tile_position=(0, r0),
                        )
                cx_sb = work.tile([128, 2 * NB], dt.float32, tag="cxs")
                nc.vector.tensor_copy(cx_sb, cx_ps)
                if not zero_bv:
                    for t in range(2):
                        nc.vector.tensor_scalar_add(
                            cx_sb[:, t * NB:(t + 1) * NB],
                            cx_sb[:, t * NB:(t + 1) * NB],
                            bv_sb[:, t:t + 1],
                        )

                # ---- out_proj + bias + LayerNorm (folded has_tokens) ----
                op_ps = pst.tile([NB, D], dt.float32, tag="tail")
                for t in range(2):
                    nc.tensor.matmul(
                        op_ps,
                        lhsT=cx_sb[:, t * NB:(t + 1) * NB],
                        rhs=woT_sb[:, t, :],
                        start=(t == 0),
                        stop=(t == 1),
                    )
                if zero_br:
                    o_ap = op_ps
                else:
                    o_sb = work.tile([NB, D], dt.float32, tag="o")
                    nc.vector.tensor_tensor(o_sb, op_ps, br_sb, op=mybir.AluOpType.add)
                    o_ap = o_sb
                st_sb = work.tile([NB, 6], dt.float32, tag="st")
                nc.vector.bn_stats(st_sb, o_ap)
                mv_sb = work.tile([NB, 2], dt.float32, tag="mv")
                nc.vector.bn_aggr(mv_sb, st_sb)
                lnv_sb = work.tile([NB, 1], dt.float32, tag="lnv")
                nc.scalar.activation(
                    lnv_sb, mv_sb[:, 1:2], func=mybir.ActivationFunctionType.Ln,
                    bias=eps_sb[0:NB],
                )
                rstd_sb = work.tile([NB, 1], dt.float32, tag="rstd")
                nc.scalar.activation(
                    rstd_sb, lnv_sb, func=mybir.ActivationFunctionType.Exp,
                    scale=-0.5,
                )
                c_sb = work.tile([NB, D], dt.float32, tag="c")
                nc.vector.tensor_scalar(
                    c_sb, o_ap, mv_sb[:, 0:1], rstd_sb,
                    op0=mybir.AluOpType.subtract, op1=mybir.AluOpType.mult,
                )
                nc.vector.tensor_tensor(c_sb, c_sb, g_sb, op=mybir.AluOpType.mult)
                nc.vector.tensor_tensor(c_sb, c_sb, b_sb, op=mybir.AluOpType.add)

                # SWDGE: keeps the result-gated store off the Sync HWDGE
                # trigger stream so later loads are never blocked behind it
                nc.gpsimd.dma_start(out=out_d[i], in_=c_sb)

    import concourse.bacc as bacc_mod

    orig_tables = bacc_mod.get_activation_tables
    bacc_mod.get_activation_tables = _patched_act_tables()
    try:
        nc.compile()
    finally:
        bacc_mod.get_activation_tables = orig_tables
    return nc


def _get_nc(zero_bv=True, zero_br=True):
    key = ("nc", zero_bv, zero_br)
    if key not in _CACHE:
        _CACHE[key] = _build_nc(zero_bv, zero_br)
    return _CACHE[key]


def _prep(inputs):
    x = np.asarray(inputs["x"], F32)
    query = np.asarray(inputs["query"], F32)
    ipw = np.asarray(inputs["in_proj_weight"], F32)
    ipb = np.asarray(inputs["in_proj_bias"], F32)
    opw = np.asarray(inputs["out_proj_weight"], F32)
    opb = np.asarray(inputs["out_proj_bias"], F32)
    gamma = np.asarray(inputs["ln_gamma"], F32)
    beta = np.asarray(inputs["ln_beta"], F32)
    mask = np.asarray(inputs["mask"]).astype(bool)
    bid = np.asarray(inputs["backend_id"]).astype(np.int32)
    nbm = int(np.asarray(inputs["n_backends_max"]))
    assert nbm == NB and x.shape == (B, L, D)

    wq, wk, wv = ipw[0:D], ipw[D:2 * D], ipw[2 * D:3 * D]
    bq, bk, bv = ipb[0:D], ipb[D:2 * D], ipb[2 * D:3 * D]

    qv = query[0, 0] @ wq.T + bq                      # (256,)
    qh = qv.reshape(H, HD)
    qw = np.einsum("hj,hjd->hd", qh, wk.reshape(H, HD, D)) * SCALE  # (8, 256)
    # bk contribution is constant per h -> cancels in softmax normalization.

    qwT = np.ascontiguousarray(qw.T * 2.0 ** QW_SHIFT).reshape(2, 128, H).astype(FP8)
    wvT = np.ascontiguousarray(wv.T).reshape(2, 128, D).astype(F32)
    woT = np.ascontiguousarray(opw.T).reshape(2, 128, D).astype(F32)
    bv_dev = np.ascontiguousarray(bv.reshape(2, 128).T).astype(F32)   # [e', t]
    br = np.ascontiguousarray(np.broadcast_to(opb, (NB, D))).astype(F32)
    ident = np.eye(128, dtype=F32)

    bm = mask[:, :, None] & (bid[:, :, None] == np.arange(NB, dtype=np.int32))
    ht = bm.any(1).astype(F32)                        # (16, 8)
    gf = np.ascontiguousarray(gamma[None, None, :] * ht[:, :, None]).astype(F32)
    bf = np.ascontiguousarray(beta[None, None, :] * ht[:, :, None]).astype(F32)
    m_host = np.ascontiguousarray(
        bm.reshape(B, NCH, 128, NB).transpose(0, 2, 1, 3)
    ).astype(BF16).reshape(B, 128, NCH * NB)

    x_bf = x.astype(BF16)
    xT = np.ascontiguousarray(x.transpose(0, 2, 1)).reshape(B, 2, 128, L).astype(FP8)
    xe = np.empty((B, 128, NCH, JW), BF16)
    xe[..., :D] = x_bf.reshape(B, NCH, 128, D).transpose(0, 2, 1, 3)
    xe[..., D] = 1
    xe = xe.reshape(B, 128, NCH * JW)

    in_maps = []
    for c in range(NCORES):
        sl = slice(BPC * c, BPC * (c + 1))
        in_maps.append({
            "xT": xT[sl], "xext": xe[sl], "m": m_host[sl],
            "qwT": qwT, "wvT": wvT, "woT": woT,
            "gf": gf[sl], "bf": bf[sl], "br": br, "bv": bv_dev,
            "ident": ident,
        })
    return in_maps


def kernel(**inputs):
    global LAST_RESULT
    in_maps = _prep(inputs)
    ipb = np.asarray(inputs["in_proj_bias"], F32)
    opb = np.asarray(inputs["out_proj_bias"], F32)
    nc = _get_nc(zero_bv=not ipb[2 * D:].any(), zero_br=not opb.any())
    res = run_bass_kernel_spmd(nc, in_maps, list(range(NCORES)))
    LAST_RESULT = res
    out = np.concatenate([res.results[c]["out"] for c in range(NCORES)], axis=0)
    return np.ascontiguousarray(out.astype(F32))


if __name__ == "__main__":
    nc = _get_nc()
    print("traced ok:", nc)



# revision 6
# speedup vs baseline: 1.3008x; 1.3008x over previous
"""Trainium2 Bass kernel for BackendQueryPooling.

Math simplifications (exact unless noted):
  - k-projection folds into the shared query:
        scores[l,h] = x[l] . qw[h],  qw[h] = (q_h @ wk_head_h) / sqrt(hd)
    (bk is constant per h and cancels in softmax).
  - v never materialized: ctx[n,h,:] = (sum_l w[n,h,l] x[l]) @ wvT_head
    since sum_l w = 1 after normalization (bv is zero here; asserted).
  - Unnormalized exp + ones-column normalization in the same matmul.
  - Host-side token compaction: tokens with mask=0 have softmax weight
    exactly 0 in the reference (logit -1e9 underflows), so they are
    dropped. Remaining tokens are SORTED by backend_id so each backend
    owns a contiguous, 128-aligned segment; padding slots have xe rows
    (incl. the ones column) set to 0 so they contribute nothing.
  - Empty backends produce 0 via gf/bf (gamma/beta pre-zeroed), as the
    reference zeroes by has_tokens.

Sharding: data-parallel over batch. 16 batches / 8 cores = 2 per core.
Both per-core batches share one [128, 257] PSUM accumulator: partition
nb*8+h where nb = batch*8 + backend. y matmuls use zero-padded [128,32]
E blocks so PE tile positions stay 32-aligned.
"""

import sys

sys.path.insert(0, "/opt/trn_rl_repo")

import numpy as np
import ml_dtypes

import concourse.bass as bass
import concourse.bacc as bacc
import concourse.tile as tile
from concourse import mybir
from concourse.bass_utils import run_bass_kernel_spmd

BF16 = ml_dtypes.bfloat16
FP8 = ml_dtypes.float8_e4m3
QW_SHIFT = 13  # qw values ~7e-4 underflow fp8; pre-scale by 2**13, undo in exp scale
F32 = np.float32

B, L, D = 16, 8192, 256
H, HD, NB = 8, 32, 8
NCORES = 8
BPC = B // NCORES          # batches per core
SCALE = 1.0 / np.sqrt(HD)
JW = D + 1                 # 257: xe row width incl. ones column
NG = BPC * NB              # 16 (batch, backend) groups per core

_CACHE = {}
LAST_RESULT = None


def _patched_act_tables():
    """Make the act-table chooser land Exp AND Ln in one set
    (natural_log_exp_and_others) instead of thrashing between
    exp_and_others and natural_log (1.28us reload per switch)."""
    from concourse.hw_specs import get_activation_tables

    AF = mybir.ActivationFunctionType

    def patched(arch):
        t = {k: set(v) for k, v in get_activation_tables(arch).items()}
        for name in t:
            if name != "natural_log_exp_and_others":
                t[name].discard(AF.Exp)
                t[name].discard(AF.Ln)
        return t

    return patched


def _build_nc(K):
    """K = chunks (of 128 tokens) per (batch, backend) segment."""
    nc = bacc.Bacc("TRN2", target_bir_lowering=False)
    dt = mybir.dt

    NCHB = NB * K              # l-chunks per batch
    NCH = BPC * NCHB           # l-chunks per core (both batches)
    LP = NCHB * 128            # padded tokens per batch

    # xe pieces (in chunks): coarse early (few DMA triggers), fine at the
    # very end so the last piece's compute tail is short
    def split_pieces(n, final):
        if not final or n <= 16:
            return [n]
        return [n - 16, 8, 4, 4]

    PIECES = []                # (batch, chunk_off_in_batch, n_chunks)
    for bi in range(BPC):
        off = 0
        ps = split_pieces(NCHB, final=(bi == BPC - 1))
        for pcs in ps:
            PIECES.append((bi, off, pcs))
            off += pcs

    xT_d = nc.dram_tensor("xT", [BPC, 2, 128, LP], dt.float8e4, kind="ExternalInput")
    xe_d = nc.dram_tensor("xe", [BPC, 128, NCHB * JW], dt.bfloat16, kind="ExternalInput")
    qwT_d = nc.dram_tensor("qwT", [2, 128, H], dt.float8e4, kind="ExternalInput")
    wvT_d = nc.dram_tensor("wvT", [2, 128, D], dt.float32, kind="ExternalInput")
    woT_d = nc.dram_tensor("woT", [2, 128, D], dt.float32, kind="ExternalInput")
    gb_d = nc.dram_tensor("gb", [2, NG, D], dt.float32, kind="ExternalInput")
    id_d = nc.dram_tensor("ident", [128, 128], dt.float32, kind="ExternalInput")
    out_d = nc.dram_tensor("out", [NG, D], dt.float32, kind="ExternalOutput")

    with tile.TileContext(nc) as tc:
        with (
            tc.tile_pool(name="consts", bufs=1) as consts,
            tc.tile_pool(name="big", bufs=1) as big,
            tc.tile_pool(name="work", bufs=2) as work,
            tc.tile_pool(name="psc", bufs=3, space="PSUM") as psc,
            tc.tile_pool(name="psy", bufs=1, space="PSUM") as psy,
            tc.tile_pool(name="pst", bufs=2, space="PSUM") as pst,
        ):
            # scores-path const first: it gates the first compute
            qwT_sb = consts.tile([128, 2, H], dt.float8e4)
            nc.scalar.dma_start(out=qwT_sb, in_=qwT_d[:].rearrange("c p h -> p c h"))

            # x loads on the Sync trigger queue, interleaved per batch so the
            # scores path (xT) and y path (xe) both stream early
            xT_sb = {}
            xe_sb = []

            def load_xT(bi):
                t = big.tile([128, 2, LP], dt.float8e4, tag=f"xT{bi}")
                nc.sync.dma_start(out=t, in_=xT_d[bi].rearrange("c p l -> p c l"))
                xT_sb[bi] = t

            def load_xe(bi, off, pcs):
                t = big.tile([128, pcs, JW], dt.bfloat16, tag=f"xe{len(xe_sb)}")
                nc.sync.dma_start(
                    out=t,
                    in_=xe_d[bi, :, off * JW:(off + pcs) * JW].rearrange(
                        "p (c j) -> p c j", j=JW
                    ),
                )
                xe_sb.append(t)

            prev_b = -1
            for bi, off, pcs in PIECES:
                if bi != prev_b:
                    load_xT(bi)
                    prev_b = bi
                load_xe(bi, off, pcs)

            # tail consts on the Scalar queue
            wvT_sb = consts.tile([128, 2, D], dt.float32)
            nc.scalar.dma_start(out=wvT_sb, in_=wvT_d[:].rearrange("c p e -> p c e"))
            woT_sb = consts.tile([128, 2, D], dt.float32)
            nc.scalar.dma_start(out=woT_sb, in_=woT_d[:].rearrange("c p e -> p c e"))
            gb_sb = consts.tile([NG, 2, D], dt.float32)
            nc.scalar.dma_start(out=gb_sb, in_=gb_d[:].rearrange("t p e -> p t e"))
            id_sb = consts.tile([128, 128], dt.float32)
            nc.scalar.dma_start(out=id_sb, in_=id_d[:])
            eps_sb = consts.tile([128, 1], dt.float32)
            nc.vector.memset(eps_sb, 1e-5)

            # zero-padded E blocks: E5[:, c, g, :] nonzero only at g = nb%4
            E5 = consts.tile([128, NCH, 4, H], dt.bfloat16)
            nc.gpsimd.memset(E5, 0.0)

            y_ps = psy.tile([128, JW], dt.float32, tag="y")

            # ---- pipelined: scores -> exp -> E5 -> y ----
            for pi, (bi, off, pcs) in enumerate(PIECES):
                gchunk = bi * NCHB + off       # global chunk index of piece start
                sc_ps = psc.tile([128, pcs * H], dt.float32, tag="sc")
                for ci in range(pcs):
                    c = off + ci
                    for dc in range(2):
                        nc.tensor.matmul(
                            sc_ps[:, ci * H:(ci + 1) * H],
                            lhsT=xT_sb[bi][:, dc, c * 128:(c + 1) * 128],
                            rhs=qwT_sb[:, dc, :],
                            start=(dc == 0),
                            stop=(dc == 1),
                        )
                E_sb = work.tile([128, pcs, H], dt.bfloat16, tag=f"E{pi % 2}")
                nc.scalar.activation(
                    out=E_sb.rearrange("p c h -> p (c h)"),
                    in_=sc_ps,
                    func=mybir.ActivationFunctionType.Exp,
                    scale=float(2.0 ** -QW_SHIFT),
                )
                # scatter into padded blocks, one copy per (backend, piece) span
                E5v = E5.rearrange("p (n k) g h -> p n k g h", k=K)
                ci = 0
                while ci < pcs:
                    c = off + ci                      # chunk in batch
                    nb = bi * NB + c // K             # global group
                    k0 = c % K
                    kn = min(K - k0, pcs - ci)        # chunks of nb in this piece
                    nc.vector.tensor_copy(
                        E5v[:, nb, k0:k0 + kn, nb % 4, :],
                        E_sb[:, ci:ci + kn, :],
                    )
                    ci += kn
                # y matmuls for this piece
                for ci in range(pcs):
                    c = off + ci
                    gc = gchunk + ci
                    nb = bi * NB + c // K
                    g32 = nb // 4
                    nc.tensor.matmul(
                        y_ps[g32 * 32:(g32 + 1) * 32, :],
                        lhsT=E5[:, gc, :, :],
                        rhs=xe_sb[pi][:, ci, :],
                        start=(gc % (4 * K) == 0),
                        stop=(gc % (4 * K) == 4 * K - 1),
                        tile_position=(0, g32 * 32),
                    )

            # ---- normalize ----
            s_sb = work.tile([128, 1], dt.float32, tag="s")
            nc.vector.tensor_scalar_add(s_sb, y_ps[:, D:JW], 1e-30)
            r_sb = work.tile([128, 1], dt.float32, tag="r")
            nc.vector.reciprocal(r_sb, s_sb)
            yn_sb = work.tile([128, D], dt.float32, tag="yn")
            nc.vector.tensor_scalar_mul(yn_sb, y_ps[:, 0:D], r_sb)

            # ---- transpose y_norm -> yT [e', dc, (nb h)] ----
            yT_sb = work.tile([128, 2, 128], dt.float32, tag="yT")
            for ec in range(2):
                tr_ps = pst.tile([128, 128], dt.float32, tag="tail")
                nc.tensor.transpose(
                    tr_ps, yn_sb[:, ec * 128:(ec + 1) * 128], id_sb
                )
                nc.vector.tensor_copy(yT_sb[:, ec, :], tr_ps)

            # ---- ctxT[e', (t, nb)] via per-head matmuls ----
            cx_ps = pst.tile([128, 2 * NG], dt.float32, tag="tail")
            yTv = yT_sb.rearrange("p c (n h2) -> p c n h2", h2=H)
            for h in range(H):
                t, r4 = divmod(h, 4)
                r0 = r4 * 32
                for dc in range(2):
                    nc.tensor.matmul(
                        cx_ps[r0:r0 + 32, t * NG:(t + 1) * NG],
                        lhsT=wvT_sb[:, dc, HD * h:HD * h + HD],
                        rhs=yTv[:, dc, :, h],
                        start=(dc == 0),
                        stop=(dc == 1),
                        tile_position=(0, r0),
                    )
            cx_sb = work.tile([128, 2 * NG], dt.float32, tag="cxs")
            nc.vector.tensor_copy(cx_sb, cx_ps)

            # ---- out_proj + LayerNorm (has_tokens folded into gf/bf) ----
            op_ps = pst.tile([NG, D], dt.float32, tag="tail")
            for t in range(2):
                nc.tensor.matmul(
                    op_ps,
                    lhsT=cx_sb[:, t * NG:(t + 1) * NG],
                    rhs=woT_sb[:, t, :],
                    start=(t == 0),
                    stop=(t == 1),
                )
            st_sb = work.tile([NG, 6], dt.float32, tag="st")
            nc.vector.bn_stats(st_sb, op_ps)
            mv_sb = work.tile([NG, 2], dt.float32, tag="mv")
            nc.vector.bn_aggr(mv_sb, st_sb)
            lnv_sb = work.tile([NG, 1], dt.float32, tag="lnv")
            nc.scalar.activation(
                lnv_sb, mv_sb[:, 1:2], func=mybir.ActivationFunctionType.Ln,
                bias=eps_sb[0:NG],
            )
            rstd_sb = work.tile([NG, 1], dt.float32, tag="rstd")
            nc.scalar.activation(
                rstd_sb, lnv_sb, func=mybir.ActivationFunctionType.Exp,
                scale=-0.5,
            )
            c_sb = work.tile([NG, D], dt.float32, tag="c")
            nc.vector.tensor_scalar(
                c_sb, op_ps, mv_sb[:, 0:1], rstd_sb,
                op0=mybir.AluOpType.subtract, op1=mybir.AluOpType.mult,
            )
            nc.vector.tensor_tensor(c_sb, c_sb, gb_sb[:, 0, :], op=mybir.AluOpType.mult)
            nc.vector.tensor_tensor(c_sb, c_sb, gb_sb[:, 1, :], op=mybir.AluOpType.add)

            # SWDGE store: keeps the output off the Sync trigger stream
            nc.gpsimd.dma_start(out=out_d[:], in_=c_sb)

    import concourse.bacc as bacc_mod

    orig_tables = bacc_mod.get_activation_tables
    bacc_mod.get_activation_tables = _patched_act_tables()
    try:
        nc.compile()
    finally:
        bacc_mod.get_activation_tables = orig_tables
    return nc


def _get_nc(K):
    key = ("nc", K)
    if key not in _CACHE:
        _CACHE[key] = _build_nc(K)
    return _CACHE[key]


def _prep(inputs):
    x = np.asarray(inputs["x"], F32)
    query = np.asarray(inputs["query"], F32)
    ipw = np.asarray(inputs["in_proj_weight"], F32)
    ipb = np.asarray(inputs["in_proj_bias"], F32)
    opw = np.asarray(inputs["out_proj_weight"], F32)
    opb = np.asarray(inputs["out_proj_bias"], F32)
    gamma = np.asarray(inputs["ln_gamma"], F32)
    beta = np.asarray(inputs["ln_beta"], F32)
    mask = np.asarray(inputs["mask"]).astype(bool)
    bid = np.asarray(inputs["backend_id"]).astype(np.int32)
    nbm = int(np.asarray(inputs["n_backends_max"]))
    assert nbm == NB and x.shape == (B, L, D)

    wq, wk, wv = ipw[0:D], ipw[D:2 * D], ipw[2 * D:3 * D]
    bq, bk, bv = ipb[0:D], ipb[D:2 * D], ipb[2 * D:3 * D]
    assert not bv.any(), "nonzero v-bias not supported by this kernel"

    qv = query[0, 0] @ wq.T + bq                      # (256,)
    qh = qv.reshape(H, HD)
    qw = np.einsum("hj,hjd->hd", qh, wk.reshape(H, HD, D)) * SCALE  # (8, 256)
    # bk contribution is constant per h -> cancels in softmax normalization.

    qwT = np.ascontiguousarray(qw.T * 2.0 ** QW_SHIFT).reshape(2, 128, H).astype(FP8)
    wvT = np.ascontiguousarray(wv.T).reshape(2, 128, D).astype(F32)
    woT = np.ascontiguousarray(opw.T).reshape(2, 128, D).astype(F32)
    ident = np.eye(128, dtype=F32)

    # segment sizes and K
    cnt = np.zeros((B, NB), np.int64)
    for b in range(B):
        cnt[b] = np.bincount(bid[b][mask[b]], minlength=NB)
    K = max(1, int(np.ceil(cnt.max() / 128)))
    LP = NB * K * 128

    # sorted + compacted token layout
    xs = np.zeros((B, LP, D), F32)
    ones = np.zeros((B, LP, 1), F32)
    for b in range(B):
        for n in range(NB):
            idx = np.flatnonzero(mask[b] & (bid[b] == n))
            o = n * K * 128
            xs[b, o:o + len(idx)] = x[b, idx]
            ones[b, o:o + len(idx)] = 1.0
    xT = np.ascontiguousarray(xs.transpose(0, 2, 1)).reshape(B, 2, 128, LP).astype(FP8)
    xe = np.empty((B, LP, JW), BF16)
    xe[..., :D] = (xs * ones).astype(BF16)
    xe[..., D] = ones[..., 0].astype(BF16)
    NCHB = NB * K
    xe = np.ascontiguousarray(
        xe.reshape(B, NCHB, 128, JW).transpose(0, 2, 1, 3)
    ).reshape(B, 128, NCHB * JW)

    # gamma/beta with has_tokens folded in; opb folded into beta is NOT
    # possible (LN is nonlinear) but opb enters before LN via matmul bias;
    # reference adds opb then LN. opb shifts mean only -> cancels in LN
    # mean subtraction... except through gamma scaling: (o+opb - mu-mean(opb))
    # = o - mu + (opb - mean(opb)). Only exactly cancels if opb uniform.
    ht = (cnt > 0).astype(F32)                        # (B, NB)
    gf = gamma[None, None, :] * ht[:, :, None]        # (B, NB, D)
    bf = beta[None, None, :] * ht[:, :, None]

    in_maps = []
    for c in range(NCORES):
        sl = slice(BPC * c, BPC * (c + 1))
        gb = np.ascontiguousarray(
            np.stack([gf[sl].reshape(NG, D), bf[sl].reshape(NG, D)])
        ).astype(F32)
        in_maps.append({
            "xT": xT[sl], "xe": xe[sl],
            "qwT": qwT, "wvT": wvT, "woT": woT,
            "gb": gb, "ident": ident,
        })
    return in_maps, K, opb


def kernel(**inputs):
    global LAST_RESULT
    in_maps, K, opb = _prep(inputs)
    assert not opb.any() or np.allclose(opb, opb[0]), \
        "non-uniform out_proj bias shifts LN mean nontrivially"
    nc = _get_nc(K)
    res = run_bass_kernel_spmd(nc, in_maps, list(range(NCORES)))
    LAST_RESULT = res
    out = np.concatenate(
        [res.results[c]["out"].reshape(BPC, NB, D) for c in range(NCORES)], axis=0
    )
    return np.ascontiguousarray(out.astype(F32))


if __name__ == "__main__":
    nc = _get_nc(5)
    print("traced ok:", nc)
